# revision 5
# baseline (speedup 1.0000x reference)
"""GCN layer kernel for Trainium2, 8-core row-parallel.

Computes out = (adj * mask + I) @ (x @ W^T) for N=8192, C_in=C_out=128.

Sharding: adj/mask row-blocks of 1024 across 8 cores; x, W replicated.
v2 layout (memory-bound; target = keep the 64MiB adj+mask stream at the
~358 GB/s per-core HBM limit with no queue gaps):
  - adj streams on the SP HWDGE ring (nc.sync), mask on the ACT HWDGE
    ring (nc.scalar) -- two independent rings interleave at SDMA packet
    granularity, so per-dma fixed costs on one ring hide under the
    other's transfers. 2MB per dma_start, 4KB descriptors.
  - x / x_own / W load on the gpsimd SWDGE queue so the HWDGE rings
    carry nothing but the big streams; the ACT ring's in-order stream is
    ONLY mask triggers + out writes (at-copies live on DVE), so compute
    can never stall the mask ring.
  - phase-0 (h = x @ W^T) emission is interleaved group-by-group ahead
    of the chunks that consume each h range; all its PSUM copies are on
    DVE too (stalls on any engine reset the PE clock ramp).
  - main matmuls keep a 512-wide moving operand (1 cycle/row fp32r).
  - finalize uses a 4-rows-per-partition permuted layout so the out
    write has 2KB descriptors; x_own/ho use the same permutation. The
    last k-chunk is split in two for a shorter tail.
"""

import numpy as np
from contextlib import ExitStack

from concourse import bass, bacc, tile, mybir
from concourse import masks
from concourse.bass_utils import run_bass_kernel_spmd

N = 8192
C = 128
NCORES = 8
R = N // NCORES          # 1024 rows per core
M_BLK = 512              # psum accumulation block (free dim of main matmul)
NBLK = R // M_BLK        # 2 m-blocks per core
S = M_BLK // 128         # 4 slabs of 128 rows per m-block
KQ = 1024                # k-chunk width per DMA iteration
NQ = N // KQ             # 8 k-chunks per m-block
NKT = N // 128           # 64 k-tiles total
XG = 2048                # x rows per phase-0 group
NXG = N // XG            # 4 groups
XT = XG // 128           # 16 x tiles per group
JF = 4                   # finalize: rows per partition (out descriptor = JF*512B)

F32 = mybir.dt.float32
F32R = mybir.dt.float32r


def build_program():
    nc = bacc.Bacc("TRN2", target_bir_lowering=False, debug=False, num_devices=NCORES)

    adj_d = nc.dram_tensor("adj", [R, N], F32, kind="ExternalInput").ap()
    mask_d = nc.dram_tensor("mask", [R, N], F32, kind="ExternalInput").ap()
    x_d = nc.dram_tensor("x", [N, C], F32, kind="ExternalInput").ap()
    xo_d = nc.dram_tensor("x_own", [R, C], F32, kind="ExternalInput").ap()
    w_d = nc.dram_tensor("w", [C, C], F32, kind="ExternalInput").ap()
    out_d = nc.dram_tensor("out", [R, C], F32, kind="ExternalOutput").ap()

    with tile.TileContext(nc) as tc, ExitStack() as ctx:
        const_pool = ctx.enter_context(tc.tile_pool(name="const", bufs=1))
        xg_pool = ctx.enter_context(tc.tile_pool(name="xg", bufs=2))
        xr_pool = ctx.enter_context(tc.tile_pool(name="xr", bufs=2))
        xt_pool = ctx.enter_context(tc.tile_pool(name="xt", bufs=3))
        h_pool = ctx.enter_context(tc.tile_pool(name="h", bufs=1))
        adj_pool = ctx.enter_context(tc.tile_pool(name="adj", bufs=2))
        mask_pool = ctx.enter_context(tc.tile_pool(name="mask", bufs=2))
        prod_pool = ctx.enter_context(tc.tile_pool(name="prod", bufs=2))
        at_pool = ctx.enter_context(tc.tile_pool(name="at", bufs=8))
        fin_pool = ctx.enter_context(tc.tile_pool(name="fin", bufs=4))
        psum_acc = ctx.enter_context(tc.tile_pool(name="pacc", bufs=2, space="PSUM"))
        psum_tr = ctx.enter_context(tc.tile_pool(name="ptr", bufs=2, space="PSUM"))
        psum_misc = ctx.enter_context(tc.tile_pool(name="pmisc", bufs=3, space="PSUM"))
        psum_fin = ctx.enter_context(tc.tile_pool(name="pfin", bufs=1, space="PSUM"))

        ident = const_pool.tile([128, 128], F32)
        masks.make_identity(nc, ident[:])
        identr = const_pool.tile([128, 128], F32R)
        nc.vector.tensor_copy(identr[:], ident[:])

        # ---- weight: W^T in fp32r, loaded on the gpsimd queue ----
        w_sb = const_pool.tile([128, C], F32)
        nc.gpsimd.dma_start(out=w_sb[:], in_=w_d[:, :])
        psum_wt = psum_misc.tile([128, 128], F32, tag="pm")
        nc.tensor.transpose(psum_wt[:], w_sb[:], ident[:])
        wtr_sb = const_pool.tile([128, C], F32R)
        nc.vector.tensor_copy(wtr_sb[:], psum_wt[:])

        # ---- x loads: 4 groups of 2048 rows, SWDGE queue ----
        xg_tiles = []
        for g in range(NXG):
            xg = xg_pool.tile([128, XT, C], F32, tag="xg")
            nc.gpsimd.dma_start(
                out=xg[:],
                in_=x_d[g * XG : (g + 1) * XG, :].rearrange("(t p) c -> p t c", p=128),
            )
            xg_tiles.append(xg)
        # x_own in the finalize permutation: [p, (blk j), c] = row blk*512+p*JF+j
        xo_sb = const_pool.tile([128, NBLK * JF, C], F32)
        for b in range(NBLK):
            nc.gpsimd.dma_start(
                out=xo_sb[:, b * JF : (b + 1) * JF, :],
                in_=xo_d[b * M_BLK : (b + 1) * M_BLK, :].rearrange(
                    "(p j) c -> p j c", p=128, j=JF
                ),
            )

        h_sb = h_pool.tile([128, NKT, C], F32R)
        ho_sb = const_pool.tile([128, NBLK * JF, C], F32)

        def h_tile_pipe(src_view, dst_view):
            # one 128-row h tile: transpose src -> xt, matmul with W^T
            psum_xt = psum_misc.tile([128, 128], F32R, tag="pm")
            nc.tensor.transpose(psum_xt[:], src_view, identr[:])
            xt_sb = xt_pool.tile([128, 128], F32R)
            nc.vector.tensor_copy(xt_sb[:], psum_xt[:])
            psum_h = psum_misc.tile([128, 128], F32, tag="pm")
            nc.tensor.matmul(psum_h[:], xt_sb[:], wtr_sb[:], start=True, stop=True)
            nc.vector.tensor_copy(dst_view, psum_h[:])

        def phase0_group(g):
            # h tiles for rows [g*2048, (g+1)*2048); all PSUM copies on DVE
            xr = xr_pool.tile([128, XT, C], F32R, tag="xr")
            nc.vector.tensor_copy(xr[:], xg_tiles[g][:])  # fp32r rounding pass
            for t in range(XT):
                h_tile_pipe(xr[:, t, :], h_sb[:, g * XT + t, :])

        def phase0_own():
            # h rows owned by this core (+I self-loop), permuted layout
            xro = xr_pool.tile([128, NBLK * JF, C], F32R, tag="xr")
            nc.vector.tensor_copy(xro[:], xo_sb[:])
            for t in range(NBLK * JF):
                h_tile_pipe(xro[:, t, :], ho_sb[:, t, :])

        # ---- main loop ----
        def do_chunk(blk, pacc, k0, kw):
            r0 = blk * M_BLK
            adj_t = adj_pool.tile([128, S, kw], F32, tag="adj")
            nc.sync.dma_start(
                out=adj_t[:],
                in_=adj_d[r0 : r0 + M_BLK, k0 : k0 + kw].rearrange(
                    "(s p) k -> p s k", p=128
                ),
            )
            mask_t = mask_pool.tile([128, S, kw], F32, tag="mask")
            nc.scalar.dma_start(
                out=mask_t[:],
                in_=mask_d[r0 : r0 + M_BLK, k0 : k0 + kw].rearrange(
                    "(s p) k -> p s k", p=128
                ),
            )
            prod_t = prod_pool.tile([128, S, kw], F32R, tag="prod")
            SUB = min(kw, 512)
            for j in range(kw // SUB):
                sl = slice(j * SUB, (j + 1) * SUB)
                nc.vector.tensor_mul(
                    prod_t[:, :, sl], adj_t[:, :, sl], mask_t[:, :, sl]
                )
            for kt in range(kw // 128):
                kg = (k0 // 128) + kt
                psum_at = psum_tr.tile([128, M_BLK], F32R)
                for s in range(S):
                    nc.tensor.transpose(
                        psum_at[:, s * 128 : (s + 1) * 128],
                        prod_t[:, s, kt * 128 : (kt + 1) * 128],
                        identr[:],
                    )
                at_sb = at_pool.tile([128, M_BLK], F32R)
                nc.vector.tensor_copy(at_sb[:], psum_at[:])
                nc.tensor.matmul(
                    pacc[:],
                    h_sb[:, kg, :],
                    at_sb[:],
                    start=(kg == 0),
                    stop=(kg == NKT - 1),
                )

        def finalize(blk, pacc):
            # out rows blk*512 + JF*p + j; 2KB out descriptors
            psum_nat = psum_fin.tile([128, JF, C], F32)
            pacc_j = pacc[:].rearrange("p (m j) -> p j m", j=JF)
            for j in range(JF):
                otj = fin_pool.tile([128, 128], F32, tag="fin_t")
                nc.vector.tensor_copy(otj[:], pacc_j[:, j, :])
                nc.tensor.transpose(psum_nat[:, j, :], otj[:], ident[:])
            out_sb = fin_pool.tile([128, JF, C], F32, tag="fin_o")
            nc.vector.tensor_add(
                out_sb[:],
                psum_nat[:],
                ho_sb[:, blk * JF : (blk + 1) * JF, :],
            )
            r0 = blk * M_BLK
            nc.scalar.dma_start(
                out=out_d[r0 : r0 + M_BLK, :].rearrange("(p j) c -> p j c", p=128),
                in_=out_sb[:],
            )

        # phase-0 group g must be emitted before chunk 2g (h k-tiles 16g..16g+15)
        p0_before = {0: 0, 1: 1, 3: 2, 5: 3}
        paccs = {}
        for blk in range(NBLK):
            pacc = psum_acc.tile([128, M_BLK], F32)
            paccs[blk] = pacc
            for q in range(NQ):
                if blk == 0 and q in p0_before:
                    phase0_group(p0_before[q])
                if blk == 0 and q == NQ - 1:
                    phase0_own()
                if blk == 1 and q == 2:
                    # blk0 finalize emitted 2 chunks into blk1 so the out
                    # write never delays the mask ring
                    finalize(0, paccs[0])
                if blk == NBLK - 1 and q == NQ - 1:
                    # split the very last chunk: shorter tail after the
                    # final bytes land
                    do_chunk(blk, pacc, q * KQ, KQ // 2)
                    do_chunk(blk, pacc, q * KQ + KQ // 2, KQ // 2)
                else:
                    do_chunk(blk, pacc, q * KQ, KQ)
        finalize(NBLK - 1, paccs[NBLK - 1])

    nc.compile()
    return nc


_NC_CACHE = None


def _get_nc():
    global _NC_CACHE
    if _NC_CACHE is None:
        _NC_CACHE = build_program()
    return _NC_CACHE


def kernel(x, adj, mask, W):
    x = np.ascontiguousarray(x, dtype=np.float32)
    adj = np.ascontiguousarray(adj, dtype=np.float32)
    mask = np.ascontiguousarray(mask, dtype=np.float32)
    W = np.ascontiguousarray(W, dtype=np.float32)

    nc = _get_nc()
    in_maps = []
    for i in range(NCORES):
        r0 = i * R
        in_maps.append(
            {
                "adj": adj[r0 : r0 + R],
                "mask": mask[r0 : r0 + R],
                "x": x,
                "x_own": x[r0 : r0 + R],
                "w": W,
            }
        )
    res = run_bass_kernel_spmd(nc, in_maps, list(range(NCORES)))
    return np.concatenate([res.results[i]["out"] for i in range(NCORES)], axis=0)
